# revision 9
# baseline (speedup 1.0000x reference)
"""Trainium2 Bass kernel for nn_CIN_81544249082266 (CIN / xDeepFM cross network).

Pure data parallel over 8 NeuronCores: each core processes 1024 of the 8192
batch rows; filters and output weights are replicated. No cross-device
communication (the host concatenates the per-core [1024] score vectors).

Math (per sample b, embedding dim d in [0,16), fields F0=39):
  layer k: z[(i,j), (b,d)] = x0[i,(b,d)] * h_k[j,(b,d)];  curr = relu(F_k^T z)
  h_{k+1} = curr rows [0:64), direct_k = remaining rows
  score[b] = sum_{m,d} direct[m,(b,d)] * (1 + w_nn[m]) + b_nn

v3 layout/engine plan (everything transposed: free axis r=(b*16+d)):
  - L0 uses only the 741 strictly-upper pairs (i<j), host-packed into 6
    partition blocks: XI/XJ [768, N] fp16 in HBM; z8 = XI*XJ on the Pool
    engine with fp8e4 output; fp8 filters (x64) via DoubleRow matmuls.
  - L1/L2: broadcast-A x replicated-h; the DVE computes z in 4-block "quad"
    instructions (h physically replicated 4x by the ACT relu so every DVE
    operand is contiguous 16-bit -> 2x DVE mode).
  - Three-stage software pipeline over 512-column steps: iteration s runs
    L0(s+2), L1(s+1), L2(s) - each stage's relu happened a full iteration
    before its consumer, so no engine waits on the relu latency chain.
  - Score contractions and reduces for step s run in iteration s+1
    (never block the in-order PE queue on ACT results).
  - All filters x64 (fp8 range); relu applies scale 1/64.
  - All loads ride the Pool SW DGE (16-engine spray) as fused descriptor
    chains: 2 per 1024-column a_all tile, 2 per 512-column XI/XJ pair.
"""

import numpy as np
import ml_dtypes
from contextlib import ExitStack

import concourse.bass as bass
import concourse.tile as tile
from concourse import bacc, mybir
from concourse.bass_utils import run_bass_kernel_spmd

F0 = 39
D = 16
B = 8192
NCORES = 8
BC = B // NCORES            # 1024 samples per core
N = BC * D                  # 16384 r-columns per core
S = 512                     # columns per pipeline step
NSTEP = N // S              # 32
NBLK = 20                   # 40*64/128 c-blocks per layer (i padded to 40)
NP0 = 6                     # L0 pair blocks (741 pairs -> 768 slots)
SC = 64.0                   # filter scaling (fp8 range), undone in relu
FP16 = mybir.dt.float16
FP32 = mybir.dt.float32
FP8 = mybir.dt.float8e4

_BUILT = None


def _build_program():
    nc = bacc.Bacc(
        "TRN2",
        target_bir_lowering=False,
        debug=False,
        num_devices=NCORES,
    )

    x2_d = nc.dram_tensor("x2", [40, N], FP16, kind="ExternalInput").ap()
    xi_d = nc.dram_tensor("xi", [NP0 * 128, N], FP16, kind="ExternalInput").ap()
    xj_d = nc.dram_tensor("xj", [NP0 * 128, N], FP16, kind="ExternalInput").ap()
    f0_d = nc.dram_tensor("f0", [128, NP0 * 128], FP8, kind="ExternalInput").ap()
    f_d = [
        nc.dram_tensor(f"f{k}", [128, NBLK * 128], FP16, kind="ExternalInput").ap()
        for k in (1, 2)
    ]
    wv_d = nc.dram_tensor("wv", [128, 2], FP16, kind="ExternalInput").ap()
    bias_d = nc.dram_tensor("bias", [1, 1], FP32, kind="ExternalInput").ap()
    out_d = nc.dram_tensor("out", [1, BC], FP32, kind="ExternalOutput").ap()

    relu = mybir.ActivationFunctionType.Relu
    mult = mybir.AluOpType.mult
    DR = mybir.MatmulPerfMode.DoubleRow

    with tile.TileContext(nc) as tc, ExitStack() as ctx:
        const = ctx.enter_context(tc.tile_pool(name="const", bufs=1))
        pool_a = ctx.enter_context(tc.tile_pool(name="a", bufs=3))
        pool_x = ctx.enter_context(tc.tile_pool(name="x", bufs=2))
        pool_z = ctx.enter_context(tc.tile_pool(name="z", bufs=3))
        pool_z8 = ctx.enter_context(tc.tile_pool(name="z8", bufs=3))
        pool_h = ctx.enter_context(tc.tile_pool(name="h", bufs=4))
        pool_dt = ctx.enter_context(tc.tile_pool(name="dt", bufs=3))
        pool_r2 = ctx.enter_context(tc.tile_pool(name="r2", bufs=2))
        ps_curr = ctx.enter_context(tc.tile_pool(name="pcur", bufs=3, space="PSUM"))
        ps_s = ctx.enter_context(tc.tile_pool(name="ps", bufs=2, space="PSUM"))

        # --- resident constants ---
        f0sb = const.tile([128, NP0 * 128], FP8, tag="f0")
        nc.sync.dma_start(f0sb[:], f0_d[:])
        fsb = []
        for k in range(2):
            f = const.tile([128, NBLK * 128], FP16, tag=f"f{k + 1}", name=f"f{k + 1}")
            nc.sync.dma_start(f[:, : NBLK * 64], f_d[k][:, : NBLK * 64])
            nc.scalar.dma_start(f[:, NBLK * 64 :], f_d[k][:, NBLK * 64 :])
            fsb.append(f)
        wv = const.tile([128, 2], FP16)
        nc.sync.dma_start(wv[:], wv_d[:])
        bias = const.tile([1, 1], FP32)
        nc.sync.dma_start(bias[:], bias_d[:])
        scores = const.tile([1, BC], FP32)

        CH = 2 * S  # a_all load granularity (1024 columns)

        def load_a(T):
            """One a_all tile covers steps 2T,2T+1; partition p = two*64+b
            holds x row 2j+two broadcast over b; 2 fused SW-DGE chains."""
            sl = slice(T * CH, (T + 1) * CH)
            a_all = pool_a.tile([128, NBLK * CH], FP16, tag="a", name=f"a_{T}")
            rows = x2_d[:, sl].rearrange("(j two) c -> two j c", two=2)
            for a in range(2):
                nc.gpsimd.dma_start(
                    a_all[64 * a : 64 * (a + 1)].rearrange(
                        "p (j c) -> p j c", j=NBLK
                    ),
                    rows[a : a + 1].to_broadcast([64, NBLK, CH]),
                )
            return a_all

        def load_x(s):
            sl = slice(s * S, (s + 1) * S)
            xia = pool_x.tile([128, NP0 * S], FP16, tag="xia", name=f"xia_{s}")
            xja = pool_x.tile([128, NP0 * S], FP16, tag="xja", name=f"xja_{s}")
            src_i = xi_d[:, sl].rearrange("(b p) c -> p b c", p=128)
            src_j = xj_d[:, sl].rearrange("(b p) c -> p b c", p=128)
            nc.gpsimd.dma_start(xia[:].rearrange("p (b c) -> p b c", b=NP0), src_i)
            nc.gpsimd.dma_start(xja[:].rearrange("p (b c) -> p b c", b=NP0), src_j)
            return xia, xja

        def relu_h(name, cur):
            """Replicated-4x hidden state: [128, 4, S], rows j doubled."""
            h_t = pool_h.tile([128, 4 * S], FP16, tag="h", name=name)
            hr = h_t[:].rearrange("p (b c) -> p b c", b=4)
            src = cur[0:64, None, :].to_broadcast([64, 4, S])
            nc.scalar.activation(hr[0:64], src, relu, scale=1.0 / SC)
            nc.scalar.activation(hr[64:128], src, relu, scale=1.0 / SC)
            return h_t

        def do_l0(s, xv):
            """L0(s): Pool fp8 z pairs + DoubleRow matmuls -> h1(s), d0."""
            xia, xja = xv
            cur = ps_curr.tile([128, S], FP32, tag="cur", name=f"cur0_{s}")
            z8s = []
            for q in range(3):
                z8 = pool_z8.tile([128, 2 * S], FP8, tag="z8", name=f"z80_{s}_{q}")
                sl2 = slice(2 * q * S, (2 * q + 2) * S)
                nc.gpsimd.tensor_tensor(
                    out=z8[:], in0=xia[:, sl2], in1=xja[:, sl2], op=mult,
                )
                z8s.append(z8)
            for q in range(3):
                nc.tensor.matmul(
                    cur[:],
                    lhsT=f0sb[:, q * 256 : (q + 1) * 256].rearrange(
                        "p (two m) -> p two m", two=2
                    ),
                    rhs=z8s[q][:].rearrange("p (two c) -> p two c", two=2),
                    start=(q == 0), stop=(q == 2),
                    perf_mode=DR,
                )
            h1 = relu_h(f"h1_{s}", cur)
            d01 = pool_dt.tile([128, S], FP16, tag="d", name=f"d_{s}")
            nc.scalar.activation(d01[0:64, :], cur[64:128, :], relu, scale=1.0 / SC)
            return h1, d01

        def layer_pass(s, layer, a_all, half, h_op, fw):
            """One 512-col layer stage: 5 DVE quads, 20 matmuls."""
            cur = ps_curr.tile([128, S], FP32, tag="cur", name=f"cur_{s}_{layer}")
            av = a_all[:].rearrange("p (j c) -> p j c", j=NBLK)
            off = half * S
            for q in range(5):
                z = pool_z.tile([128, 4 * S], FP16, tag="z", name=f"z_{s}_{layer}_{q}")
                nc.vector.tensor_tensor(
                    out=z[:].rearrange("p (b c) -> p b c", b=4),
                    in0=av[:, 4 * q : 4 * q + 4, off : off + S],
                    in1=h_op[:].rearrange("p (b c) -> p b c", b=4),
                    op=mult,
                )
                for j in range(4):
                    k = 4 * q + j
                    nc.tensor.matmul(
                        cur[:],
                        lhsT=fw[:, k * 128 : (k + 1) * 128],
                        rhs=z[:, j * S : (j + 1) * S],
                        start=(k == 0),
                        stop=(k == NBLK - 1),
                    )
            return cur

        def emit_scores(ps_, pd01, pr2):
            sab = ps_s.tile([1, S], FP32, tag="sab", name=f"sab_{ps_}")
            nc.tensor.matmul(
                sab[:], lhsT=wv[:, 0:1], rhs=pd01[:], start=True, stop=False,
            )
            nc.tensor.matmul(
                sab[:], lhsT=wv[:, 1:2], rhs=pr2[:], start=False, stop=True,
            )
            nc.vector.tensor_reduce(
                out=scores[0:1, ps_ * 32 : ps_ * 32 + 32],
                in_=sab[0:1, :].rearrange("p (g x) -> p g x", x=D),
                axis=mybir.AxisListType.X,
                op=mybir.AluOpType.add,
            )

        # ---- pipeline: iteration s runs L0(s+2), L1(s+1), scores(s-1), L2(s)
        a_tiles = {T: load_a(T) for T in range(3)}
        x_tiles = {s: load_x(s) for s in range(3)}
        l0_out = {0: do_l0(0, x_tiles.pop(0)), 1: do_l0(1, x_tiles.pop(1))}

        h1_0, d01_0 = l0_out.pop(0)
        cur1 = layer_pass(0, 1, a_tiles[0], 0, h1_0, fsb[0])
        h2s = {0: relu_h("h2_0", cur1)}
        nc.scalar.activation(d01_0[64:128, :], cur1[64:128, :], relu, scale=1.0 / SC)
        d01_done = {0: d01_0}

        pending = None
        for s in range(NSTEP):
            if s % 2 == 0 and s // 2 + 3 < NSTEP // 2:
                a_tiles[s // 2 + 3] = load_a(s // 2 + 3)
            if s + 3 < NSTEP:
                x_tiles[s + 3] = load_x(s + 3)
            if s + 2 < NSTEP:
                l0_out[s + 2] = do_l0(s + 2, x_tiles.pop(s + 2))
            if s + 1 < NSTEP:
                h1, d01 = l0_out.pop(s + 1)
                cur1 = layer_pass(
                    s + 1, 1, a_tiles[(s + 1) // 2], (s + 1) % 2, h1, fsb[0]
                )
                h2s[s + 1] = relu_h(f"h2_{s + 1}", cur1)
                nc.scalar.activation(
                    d01[64:128, :], cur1[64:128, :], relu, scale=1.0 / SC
                )
                d01_done[s + 1] = d01
            if pending is not None:
                emit_scores(*pending)
            cur2 = layer_pass(s, 2, a_tiles[s // 2], s % 2, h2s.pop(s), fsb[1])
            r2 = pool_r2.tile([128, S], FP16, tag="r2", name=f"r2_{s}")
            nc.scalar.activation(r2[:], cur2[:], relu, scale=1.0 / SC)
            if s % 2 == 1:
                del a_tiles[s // 2]
            pending = (s, d01_done.pop(s), r2)

        emit_scores(*pending)
        nc.vector.tensor_scalar_add(scores[:], scores[:], bias[0:1, 0:1])
        nc.sync.dma_start(out_d[:], scores[:])

    nc.compile()
    return nc


def _prep_inputs(nn_input, f0, f1, f2, w_nn, b_nn):
    """Host-side preprocessing into the kernel's layouts."""
    nn_input = np.asarray(nn_input, dtype=np.float32)
    f0 = np.asarray(f0, dtype=np.float32)
    f1 = np.asarray(f1, dtype=np.float32)
    f2 = np.asarray(f2, dtype=np.float32)
    w_nn = np.asarray(w_nn, dtype=np.float32).reshape(-1)
    b_nn = np.asarray(b_nn, dtype=np.float32).reshape(-1)

    # L1/L2 filters: [39*64, 128] i-major, x64, fp16 lhsT blocks
    def pack(f):
        out = np.zeros((NBLK * 128, 128), np.float32)
        out[: F0 * 64] = SC * f
        blocks = out.reshape(NBLK, 128, 128).transpose(1, 0, 2)
        return np.ascontiguousarray(blocks.reshape(128, NBLK * 128)).astype(
            np.float16
        )

    f1p, f2p = pack(f1), pack(f2)

    # L0: strictly-upper pairs packed; filter x2 (sym) x64, fp8 lhsT blocks
    iu, ju = np.triu_indices(F0, k=1)
    f0r = f0.reshape(F0, F0, 128)
    w0 = np.zeros((NP0 * 128, 128), np.float32)
    w0[: len(iu)] = 2.0 * SC * f0r[iu, ju]
    w0b = w0.reshape(NP0, 128, 128).transpose(1, 0, 2).reshape(128, NP0 * 128)
    f0p = np.ascontiguousarray(w0b).astype(ml_dtypes.float8_e4m3)

    wv = np.zeros((128, 2), np.float32)
    wv[0:64, 0] = 1.0 + w_nn[0:64]
    wv[64:128, 0] = 1.0 + w_nn[64:128]
    wv[:, 1] = 1.0 + w_nn[128:256]
    wv = wv.astype(np.float16)
    bias = b_nn.reshape(1, 1).astype(np.float32)

    x0 = nn_input.reshape(B, F0, D)
    in_maps = []
    for cidx in range(NCORES):
        xc = x0[cidx * BC : (cidx + 1) * BC]            # [BC, 39, 16]
        xt = xc.transpose(1, 0, 2).reshape(F0, N).astype(np.float16)
        x2h = np.zeros((40, N), np.float16)
        x2h[:F0] = xt
        xi = np.zeros((NP0 * 128, N), np.float16)
        xj = np.zeros((NP0 * 128, N), np.float16)
        xi[: len(iu)] = xt[iu]
        xj[: len(ju)] = xt[ju]
        in_maps.append(
            {"x2": x2h, "xi": xi, "xj": xj, "f0": f0p,
             "f1": f1p, "f2": f2p, "wv": wv, "bias": bias}
        )
    return in_maps


def _run(inputs, trace=False, trace_kwargs=None):
    global _BUILT
    if _BUILT is None:
        _BUILT = _build_program()
    nc = _BUILT
    in_maps = _prep_inputs(**inputs)
    res = run_bass_kernel_spmd(
        nc,
        in_maps,
        core_ids=list(range(NCORES)),
        trace=trace,
        **(trace_kwargs or {}),
    )
    out = np.concatenate(
        [res.results[c]["out"].reshape(BC) for c in range(NCORES)]
    )
    return out.reshape(B, 1).astype(np.float32), res


def kernel(**inputs):
    out, _ = _run(inputs)
    return out


# revision 10
# speedup vs baseline: 1.1320x; 1.1320x over previous
"""Trainium2 Bass kernel for nn_CIN_81544249082266 (CIN / xDeepFM cross network).

Pure data parallel over 8 NeuronCores: each core processes 1024 of the 8192
batch rows; filters and output weights are replicated. No cross-device
communication (the host concatenates the per-core [1024] score vectors).

Math (per sample b, embedding dim d in [0,16), fields F0=39):
  layer k: z[(i,j), (b,d)] = x0[i,(b,d)] * h_k[j,(b,d)];  curr = relu(F_k^T z)
  h_{k+1} = curr rows [0:64), direct_k = remaining rows
  score[b] = sum_{m,d} direct[m,(b,d)] * (1 + w_nn[m]) + b_nn

v3 layout/engine plan (everything transposed: free axis r=(b*16+d)):
  - L0 uses only the 741 strictly-upper pairs (i<j), host-packed into 6
    partition blocks: XI/XJ [768, N] fp16 in HBM; z8 = XI*XJ on the Pool
    engine with fp8e4 output; fp8 filters (x64) via DoubleRow matmuls.
  - L1/L2: broadcast-A x replicated-h; the DVE computes z in 4-block "quad"
    instructions (h physically replicated 4x by the ACT relu so every DVE
    operand is contiguous 16-bit -> 2x DVE mode).
  - Three-stage software pipeline over 512-column steps: iteration s runs
    L0(s+2), L1(s+1), L2(s) - each stage's relu happened a full iteration
    before its consumer, so no engine waits on the relu latency chain.
  - Score contractions and reduces for step s run in iteration s+1
    (never block the in-order PE queue on ACT results).
  - All filters x64 (fp8 range); relu applies scale 1/64.
  - All loads ride the Pool SW DGE (16-engine spray) as fused descriptor
    chains: 2 per 1024-column a_all tile, 2 per 512-column XI/XJ pair.
"""

import numpy as np
import ml_dtypes
from contextlib import ExitStack

import concourse.bass as bass
import concourse.tile as tile
from concourse import bacc, mybir
from concourse.bass_utils import run_bass_kernel_spmd

F0 = 39
D = 16
B = 8192
NCORES = 8
BC = B // NCORES            # 1024 samples per core
N = BC * D                  # 16384 r-columns per core
S = 512                     # columns per pipeline step
NSTEP = N // S              # 32
NBLK = 20                   # 40*64/128 c-blocks per layer (i padded to 40)
NP0 = 6                     # L0 pair blocks (741 pairs -> 768 slots)
SC = 64.0                   # filter scaling (fp8 range), undone in relu
FP16 = mybir.dt.float16
FP32 = mybir.dt.float32
FP8 = mybir.dt.float8e4

_BUILT = None


def _build_program():
    nc = bacc.Bacc(
        "TRN2",
        target_bir_lowering=False,
        debug=False,
        num_devices=NCORES,
    )

    x2_d = nc.dram_tensor("x2", [40, N], FP16, kind="ExternalInput").ap()
    xi_d = nc.dram_tensor("xi", [NP0 * 128, N], FP16, kind="ExternalInput").ap()
    xj_d = nc.dram_tensor("xj", [NP0 * 128, N], FP16, kind="ExternalInput").ap()
    f0_d = nc.dram_tensor("f0", [128, NP0 * 128], FP8, kind="ExternalInput").ap()
    f_d = [
        nc.dram_tensor(f"f{k}", [128, NBLK * 128], FP16, kind="ExternalInput").ap()
        for k in (1, 2)
    ]
    wv_d = nc.dram_tensor("wv", [128, 2], FP16, kind="ExternalInput").ap()
    bias_d = nc.dram_tensor("bias", [1, 1], FP32, kind="ExternalInput").ap()
    out_d = nc.dram_tensor("out", [1, BC], FP32, kind="ExternalOutput").ap()

    relu = mybir.ActivationFunctionType.Relu
    mult = mybir.AluOpType.mult
    DR = mybir.MatmulPerfMode.DoubleRow

    with tile.TileContext(nc) as tc, ExitStack() as ctx:
        const = ctx.enter_context(tc.tile_pool(name="const", bufs=1))
        pool_a = ctx.enter_context(tc.tile_pool(name="a", bufs=3))
        pool_x = ctx.enter_context(tc.tile_pool(name="x", bufs=2))
        pool_z = ctx.enter_context(tc.tile_pool(name="z", bufs=4))
        pool_z8 = ctx.enter_context(tc.tile_pool(name="z8", bufs=3))
        pool_h = ctx.enter_context(tc.tile_pool(name="h", bufs=4))
        pool_dt = ctx.enter_context(tc.tile_pool(name="dt", bufs=3))
        pool_r2 = ctx.enter_context(tc.tile_pool(name="r2", bufs=2))
        ps_curr = ctx.enter_context(tc.tile_pool(name="pcur", bufs=3, space="PSUM"))
        ps_s = ctx.enter_context(tc.tile_pool(name="ps", bufs=2, space="PSUM"))

        # --- resident constants ---
        f0sb = const.tile([128, NP0 * 128], FP8, tag="f0")
        nc.sync.dma_start(f0sb[:], f0_d[:])
        fsb = []
        for k in range(2):
            f = const.tile([128, NBLK * 128], FP16, tag=f"f{k + 1}", name=f"f{k + 1}")
            nc.sync.dma_start(f[:, : NBLK * 64], f_d[k][:, : NBLK * 64])
            nc.scalar.dma_start(f[:, NBLK * 64 :], f_d[k][:, NBLK * 64 :])
            fsb.append(f)
        wv = const.tile([128, 2], FP16)
        nc.sync.dma_start(wv[:], wv_d[:])
        bias = const.tile([1, 1], FP32)
        nc.sync.dma_start(bias[:], bias_d[:])
        scores = const.tile([1, BC], FP32)

        CH = 2 * S  # a_all load granularity (1024 columns)

        def load_a(T):
            """One a_all tile covers steps 2T,2T+1; partition p = two*64+b
            holds x row 2j+two broadcast over b; 2 fused SW-DGE chains."""
            sl = slice(T * CH, (T + 1) * CH)
            a_all = pool_a.tile([128, NBLK * CH], FP16, tag="a", name=f"a_{T}")
            rows = x2_d[:, sl].rearrange("(j two) c -> two j c", two=2)
            for a in range(2):
                nc.gpsimd.dma_start(
                    a_all[64 * a : 64 * (a + 1)].rearrange(
                        "p (j c) -> p j c", j=NBLK
                    ),
                    rows[a : a + 1].to_broadcast([64, NBLK, CH]),
                )
            return a_all

        def load_x(s):
            sl = slice(s * S, (s + 1) * S)
            xia = pool_x.tile([128, NP0 * S], FP16, tag="xia", name=f"xia_{s}")
            xja = pool_x.tile([128, NP0 * S], FP16, tag="xja", name=f"xja_{s}")
            src_i = xi_d[:, sl].rearrange("(b p) c -> p b c", p=128)
            src_j = xj_d[:, sl].rearrange("(b p) c -> p b c", p=128)
            nc.gpsimd.dma_start(xia[:].rearrange("p (b c) -> p b c", b=NP0), src_i)
            nc.gpsimd.dma_start(xja[:].rearrange("p (b c) -> p b c", b=NP0), src_j)
            return xia, xja

        def relu_h(name, cur):
            """Replicated-4x hidden state: [128, 4, S], rows j doubled."""
            h_t = pool_h.tile([128, 4 * S], FP16, tag="h", name=name)
            hr = h_t[:].rearrange("p (b c) -> p b c", b=4)
            src = cur[0:64, None, :].to_broadcast([64, 4, S])
            nc.scalar.activation(hr[0:64], src, relu, scale=1.0 / SC)
            nc.scalar.activation(hr[64:128], src, relu, scale=1.0 / SC)
            return h_t

        def do_l0(s, xv):
            """L0(s): Pool fp8 z pairs + DoubleRow matmuls -> h1(s), d0."""
            xia, xja = xv
            cur = ps_curr.tile([128, S], FP32, tag="cur", name=f"cur0_{s}")
            z8s = []
            for q in range(3):
                z8 = pool_z8.tile([128, 2 * S], FP8, tag="z8", name=f"z80_{s}_{q}")
                sl2 = slice(2 * q * S, (2 * q + 2) * S)
                nc.gpsimd.tensor_tensor(
                    out=z8[:], in0=xia[:, sl2], in1=xja[:, sl2], op=mult,
                )
                z8s.append(z8)
            for q in range(3):
                nc.tensor.matmul(
                    cur[:],
                    lhsT=f0sb[:, q * 256 : (q + 1) * 256].rearrange(
                        "p (two m) -> p two m", two=2
                    ),
                    rhs=z8s[q][:].rearrange("p (two c) -> p two c", two=2),
                    start=(q == 0), stop=(q == 2),
                    perf_mode=DR,
                )
            h1 = relu_h(f"h1_{s}", cur)
            d01 = pool_dt.tile([128, S], FP16, tag="d", name=f"d_{s}")
            nc.scalar.activation(d01[0:64, :], cur[64:128, :], relu, scale=1.0 / SC)
            return h1, d01

        def layer_pass(s, layer, a_all, half, h_op, fw):
            """One 512-col layer stage: 5 DVE quads, 20 matmuls."""
            cur = ps_curr.tile([128, S], FP32, tag="cur", name=f"cur_{s}_{layer}")
            av = a_all[:].rearrange("p (j c) -> p j c", j=NBLK)
            off = half * S
            for q in range(5):
                z = pool_z.tile([128, 4 * S], FP16, tag="z", name=f"z_{s}_{layer}_{q}")
                nc.vector.tensor_tensor(
                    out=z[:].rearrange("p (b c) -> p b c", b=4),
                    in0=av[:, 4 * q : 4 * q + 4, off : off + S],
                    in1=h_op[:].rearrange("p (b c) -> p b c", b=4),
                    op=mult,
                )
                for j in range(4):
                    k = 4 * q + j
                    nc.tensor.matmul(
                        cur[:],
                        lhsT=fw[:, k * 128 : (k + 1) * 128],
                        rhs=z[:, j * S : (j + 1) * S],
                        start=(k == 0),
                        stop=(k == NBLK - 1),
                    )
            return cur

        def emit_scores(ps_, pd01, pr2):
            sab = ps_s.tile([1, S], FP32, tag="sab", name=f"sab_{ps_}")
            nc.tensor.matmul(
                sab[:], lhsT=wv[:, 0:1], rhs=pd01[:], start=True, stop=False,
            )
            nc.tensor.matmul(
                sab[:], lhsT=wv[:, 1:2], rhs=pr2[:], start=False, stop=True,
            )
            nc.vector.tensor_reduce(
                out=scores[0:1, ps_ * 32 : ps_ * 32 + 32],
                in_=sab[0:1, :].rearrange("p (g x) -> p g x", x=D),
                axis=mybir.AxisListType.X,
                op=mybir.AluOpType.add,
            )

        # ---- pipeline: iteration s runs L0(s+2), L1(s+1), scores(s-1), L2(s)
        a_tiles = {T: load_a(T) for T in range(3)}
        x_tiles = {s: load_x(s) for s in range(3)}
        l0_out = {0: do_l0(0, x_tiles.pop(0)), 1: do_l0(1, x_tiles.pop(1))}

        h1_0, d01_0 = l0_out.pop(0)
        cur1 = layer_pass(0, 1, a_tiles[0], 0, h1_0, fsb[0])
        h2s = {0: relu_h("h2_0", cur1)}
        nc.scalar.activation(d01_0[64:128, :], cur1[64:128, :], relu, scale=1.0 / SC)
        d01_done = {0: d01_0}

        pending = None
        for s in range(NSTEP):
            if s + 2 < NSTEP:
                l0_out[s + 2] = do_l0(s + 2, x_tiles.pop(s + 2))
            if s >= 2 and s % 2 == 0 and s // 2 + 2 < NSTEP // 2:
                a_tiles[s // 2 + 2] = load_a(s // 2 + 2)
            if s + 3 < NSTEP:
                x_tiles[s + 3] = load_x(s + 3)
            if s + 1 < NSTEP:
                h1, d01 = l0_out.pop(s + 1)
                cur1 = layer_pass(
                    s + 1, 1, a_tiles[(s + 1) // 2], (s + 1) % 2, h1, fsb[0]
                )
                h2s[s + 1] = relu_h(f"h2_{s + 1}", cur1)
                nc.scalar.activation(
                    d01[64:128, :], cur1[64:128, :], relu, scale=1.0 / SC
                )
                d01_done[s + 1] = d01
            if pending is not None:
                emit_scores(*pending)
            cur2 = layer_pass(s, 2, a_tiles[s // 2], s % 2, h2s.pop(s), fsb[1])
            r2 = pool_r2.tile([128, S], FP16, tag="r2", name=f"r2_{s}")
            nc.scalar.activation(r2[:], cur2[:], relu, scale=1.0 / SC)
            if s % 2 == 1:
                del a_tiles[s // 2]
            pending = (s, d01_done.pop(s), r2)

        emit_scores(*pending)
        nc.vector.tensor_scalar_add(scores[:], scores[:], bias[0:1, 0:1])
        nc.sync.dma_start(out_d[:], scores[:])

    nc.compile()
    return nc


def _prep_inputs(nn_input, f0, f1, f2, w_nn, b_nn):
    """Host-side preprocessing into the kernel's layouts."""
    nn_input = np.asarray(nn_input, dtype=np.float32)
    f0 = np.asarray(f0, dtype=np.float32)
    f1 = np.asarray(f1, dtype=np.float32)
    f2 = np.asarray(f2, dtype=np.float32)
    w_nn = np.asarray(w_nn, dtype=np.float32).reshape(-1)
    b_nn = np.asarray(b_nn, dtype=np.float32).reshape(-1)

    # L1/L2 filters: [39*64, 128] i-major, x64, fp16 lhsT blocks
    def pack(f):
        out = np.zeros((NBLK * 128, 128), np.float32)
        out[: F0 * 64] = SC * f
        blocks = out.reshape(NBLK, 128, 128).transpose(1, 0, 2)
        return np.ascontiguousarray(blocks.reshape(128, NBLK * 128)).astype(
            np.float16
        )

    f1p, f2p = pack(f1), pack(f2)

    # L0: strictly-upper pairs packed; filter x2 (sym) x64, fp8 lhsT blocks
    iu, ju = np.triu_indices(F0, k=1)
    f0r = f0.reshape(F0, F0, 128)
    w0 = np.zeros((NP0 * 128, 128), np.float32)
    w0[: len(iu)] = 2.0 * SC * f0r[iu, ju]
    w0b = w0.reshape(NP0, 128, 128).transpose(1, 0, 2).reshape(128, NP0 * 128)
    f0p = np.ascontiguousarray(w0b).astype(ml_dtypes.float8_e4m3)

    wv = np.zeros((128, 2), np.float32)
    wv[0:64, 0] = 1.0 + w_nn[0:64]
    wv[64:128, 0] = 1.0 + w_nn[64:128]
    wv[:, 1] = 1.0 + w_nn[128:256]
    wv = wv.astype(np.float16)
    bias = b_nn.reshape(1, 1).astype(np.float32)

    x0 = nn_input.reshape(B, F0, D)
    in_maps = []
    for cidx in range(NCORES):
        xc = x0[cidx * BC : (cidx + 1) * BC]            # [BC, 39, 16]
        xt = xc.transpose(1, 0, 2).reshape(F0, N).astype(np.float16)
        x2h = np.zeros((40, N), np.float16)
        x2h[:F0] = xt
        xi = np.zeros((NP0 * 128, N), np.float16)
        xj = np.zeros((NP0 * 128, N), np.float16)
        xi[: len(iu)] = xt[iu]
        xj[: len(ju)] = xt[ju]
        in_maps.append(
            {"x2": x2h, "xi": xi, "xj": xj, "f0": f0p,
             "f1": f1p, "f2": f2p, "wv": wv, "bias": bias}
        )
    return in_maps


def _run(inputs, trace=False, trace_kwargs=None):
    global _BUILT
    if _BUILT is None:
        _BUILT = _build_program()
    nc = _BUILT
    in_maps = _prep_inputs(**inputs)
    res = run_bass_kernel_spmd(
        nc,
        in_maps,
        core_ids=list(range(NCORES)),
        trace=trace,
        **(trace_kwargs or {}),
    )
    out = np.concatenate(
        [res.results[c]["out"].reshape(BC) for c in range(NCORES)]
    )
    return out.reshape(B, 1).astype(np.float32), res


def kernel(**inputs):
    out, _ = _run(inputs)
    return out


# revision 11
# speedup vs baseline: 1.1599x; 1.0246x over previous
"""Baseline reconstruction for device health check."""

import numpy as np
from contextlib import ExitStack

import concourse.bass as bass
import concourse.tile as tile
from concourse import bacc, mybir
from concourse.bass_utils import run_bass_kernel_spmd

F0 = 39
D = 16
B = 8192
NCORES = 8
BC = B // NCORES
N = BC * D
CH = 1024
NCHUNK = N // CH
NBLK = 20
NBLK_L0 = 19
FP16 = mybir.dt.float16
FP32 = mybir.dt.float32

_BUILT = None


def _build_program():
    nc = bacc.Bacc(
        "TRN2",
        target_bir_lowering=False,
        debug=False,
        num_devices=NCORES,
    )

    x2_d = nc.dram_tensor("x2", [64, N], FP16, kind="ExternalInput").ap()
    f_d = [
        nc.dram_tensor(f"f{k}", [128, NBLK * 128], FP16, kind="ExternalInput").ap()
        for k in range(3)
    ]
    wv_d = nc.dram_tensor("wv", [128, 3], FP16, kind="ExternalInput").ap()
    bias_d = nc.dram_tensor("bias", [1, 1], FP32, kind="ExternalInput").ap()
    out_d = nc.dram_tensor("out", [1, BC], FP32, kind="ExternalOutput").ap()

    relu = mybir.ActivationFunctionType.Relu

    with tile.TileContext(nc) as tc, ExitStack() as ctx:
        const = ctx.enter_context(tc.tile_pool(name="const", bufs=1))
        pool_a = ctx.enter_context(tc.tile_pool(name="a", bufs=64))
        pool_x = ctx.enter_context(tc.tile_pool(name="x", bufs=5))
        pool_z = ctx.enter_context(tc.tile_pool(name="z", bufs=8))
        pool_h = ctx.enter_context(tc.tile_pool(name="h", bufs=3))
        pool_dt = ctx.enter_context(tc.tile_pool(name="dt", bufs=5))
        pool_r2 = ctx.enter_context(tc.tile_pool(name="r2", bufs=3))
        ps_curr = ctx.enter_context(tc.tile_pool(name="pcur", bufs=3, space="PSUM"))
        ps_s = ctx.enter_context(tc.tile_pool(name="ps", bufs=2, space="PSUM"))

        fsb = []
        for k in range(3):
            f = const.tile([128, NBLK * 128], FP16, tag=f"f{k}", name=f"f{k}")
            nc.gpsimd.dma_start(f[:, : NBLK * 64], f_d[k][:, : NBLK * 64])
            nc.gpsimd.dma_start(f[:, NBLK * 64 :], f_d[k][:, NBLK * 64 :])
            fsb.append(f)
        wv = const.tile([128, 3], FP16)
        nc.sync.dma_start(wv[:], wv_d[:])
        bias = const.tile([1, 1], FP32)
        nc.sync.dma_start(bias[:], bias_d[:])
        scores = const.tile([1, BC], FP32)

        def load_chunk(c):
            sl = slice(c * CH, (c + 1) * CH)
            xc = pool_x.tile([128, CH], FP16, tag="xc", name=f"xc_{c}")
            nc.sync.dma_start(xc[0:64, :], x2_d[:, sl])
            nc.scalar.dma_start(xc[64:128, :], x2_d[:, sl])
            a_tiles = []
            for k in range(NBLK):
                a = pool_a.tile([128, CH], FP16, tag="a", name=f"a_{c}_{k}")
                src = x2_d[2 * k : 2 * k + 2, None, sl].to_broadcast([2, 64, CH])
                eng = nc.scalar if k == 18 else (nc.sync if k == 19 else nc.gpsimd)
                eng.dma_start(a[:], src)
                a_tiles.append(a)
            return a_tiles, xc

        def layer_pass(c, layer, a_tiles, b_op, fw):
            nblk = NBLK_L0 if layer == 0 else NBLK
            cur = ps_curr.tile([128, CH], FP32, tag="cur", name=f"cur_{c}_{layer}")
            for k in range(nblk):
                z = pool_z.tile([128, CH], FP16, tag="z", name=f"z_{c}_{layer}_{k}")
                nc.vector.tensor_tensor(
                    out=z[:], in0=a_tiles[k][:], in1=b_op[:],
                    op=mybir.AluOpType.mult,
                )
                for sgn in range(2):
                    ssl = slice(sgn * 512, (sgn + 1) * 512)
                    nc.tensor.matmul(
                        cur[:, ssl],
                        lhsT=fw[:, k * 128 : (k + 1) * 128],
                        rhs=z[:, ssl],
                        start=(k == 0),
                        stop=(k == nblk - 1),
                    )
            return cur

        def score_mms(sab, layer, rhs_t, rhs_k):
            for sgn in range(2):
                ssl = slice(sgn * 512, (sgn + 1) * 512)
                nc.tensor.matmul(
                    sab[32 * sgn : 32 * sgn + 1, :],
                    lhsT=wv[0:rhs_k, layer : layer + 1],
                    rhs=rhs_t[0:rhs_k, ssl],
                    start=(layer == 0), stop=(layer == 2),
                    tile_position=(0, 32 * sgn),
                )

        def do_l0(c, a_tiles, xc):
            cur = layer_pass(c, 0, a_tiles, xc, fsb[0])
            h_t = pool_h.tile([128, CH], FP16, tag="h", name=f"h_{c}")
            d_t = pool_dt.tile([64, CH], FP16, tag="d", name=f"d_{c}")
            nc.scalar.activation(h_t[0:64, :], cur[0:64, :], relu)
            nc.scalar.activation(h_t[64:128, :], cur[0:64, :], relu)
            nc.scalar.activation(d_t[:], cur[64:128, :], relu)
            return h_t, d_t

        chunks = {}
        for cc in range(3):
            chunks[cc] = load_chunk(cc)
        state = do_l0(0, chunks[0][0], chunks[0][1])

        def emit_reduces(t, sab_t):
            for sgn in range(2):
                off = t * (CH // D) + sgn * 32
                nc.vector.tensor_reduce(
                    out=scores[0:1, off : off + 32],
                    in_=sab_t[32 * sgn : 32 * sgn + 1, :].rearrange(
                        "p (g x) -> p g x", x=D
                    ),
                    axis=mybir.AxisListType.X,
                    op=mybir.AluOpType.add,
                )

        pending_reduce = None
        for t in range(NCHUNK):
            if t + 3 < NCHUNK:
                chunks[t + 3] = load_chunk(t + 3)
            a_tiles, _ = chunks[t]
            h1, d_t = state
            cur1 = layer_pass(t, 1, a_tiles, h1, fsb[1])
            if pending_reduce is not None:
                pt, pd0, pd1, pr2 = pending_reduce
                psab = ps_s.tile([33, 512], FP32, tag="sab", name=f"sab_{pt}")
                score_mms(psab, 0, pd0, 64)
                score_mms(psab, 1, pd1, 64)
                score_mms(psab, 2, pr2, 128)
                emit_reduces(pt, psab)
            h2 = pool_h.tile([128, CH], FP16, tag="h", name=f"h2_{t}")
            d1 = pool_dt.tile([64, CH], FP16, tag="d", name=f"d1_{t}")
            nc.scalar.activation(h2[0:64, :], cur1[0:64, :], relu)
            nc.scalar.activation(h2[64:128, :], cur1[0:64, :], relu)
            nc.scalar.activation(d1[:], cur1[64:128, :], relu)
            if t + 1 < NCHUNK:
                state = do_l0(t + 1, chunks[t + 1][0], chunks[t + 1][1])
            cur2 = layer_pass(t, 2, a_tiles, h2, fsb[2])
            r2 = pool_r2.tile([128, CH], FP16, tag="r2", name=f"r2_{t}")
            nc.scalar.activation(r2[:], cur2[:], relu)
            del chunks[t]
            pending_reduce = (t, d_t, d1, r2)

        pt, pd0, pd1, pr2 = pending_reduce
        psab = ps_s.tile([33, 512], FP32, tag="sab", name=f"sab_{pt}")
        score_mms(psab, 0, pd0, 64)
        score_mms(psab, 1, pd1, 64)
        score_mms(psab, 2, pr2, 128)
        emit_reduces(pt, psab)
        nc.vector.tensor_scalar_add(scores[:], scores[:], bias[0:1, 0:1])
        nc.sync.dma_start(out_d[:], scores[:])

    nc.compile()
    return nc


def _prep_inputs(nn_input, f0, f1, f2, w_nn, b_nn):
    nn_input = np.asarray(nn_input, dtype=np.float32)
    f0 = np.asarray(f0, dtype=np.float32)
    f1 = np.asarray(f1, dtype=np.float32)
    f2 = np.asarray(f2, dtype=np.float32)
    w_nn = np.asarray(w_nn, dtype=np.float32).reshape(-1)
    b_nn = np.asarray(b_nn, dtype=np.float32).reshape(-1)

    def pack(fp):
        blocks = fp.reshape(NBLK, 128, 128)
        return np.ascontiguousarray(
            blocks.transpose(1, 0, 2).reshape(128, NBLK * 128)
        ).astype(np.float16)

    f0p = np.zeros((40, 64, 128), np.float32)
    f0r = f0.reshape(F0, F0, 128)
    iu, ju = np.triu_indices(F0, k=1)
    f0p[iu, ju] = 2.0 * f0r[iu, ju]
    f0p = pack(f0p.reshape(2560, 128))

    def padf(f):
        out = np.zeros((2560, 128), np.float32)
        out[: F0 * 64] = f
        return pack(out)

    f1p, f2p = padf(f1), padf(f2)

    wv = np.zeros((128, 3), np.float32)
    wv[0:64, 0] = 1.0 + w_nn[0:64]
    wv[0:64, 1] = 1.0 + w_nn[64:128]
    wv[:, 2] = 1.0 + w_nn[128:256]
    wv = wv.astype(np.float16)
    bias = b_nn.reshape(1, 1).astype(np.float32)

    x0 = nn_input.reshape(B, F0, D)
    in_maps = []
    for cidx in range(NCORES):
        xc = x0[cidx * BC : (cidx + 1) * BC]
        xt = xc.transpose(1, 0, 2).reshape(F0, N)
        x2h = np.zeros((64, N), np.float16)
        x2h[:F0] = xt.astype(np.float16)
        in_maps.append(
            {"x2": x2h, "f0": f0p, "f1": f1p, "f2": f2p, "wv": wv, "bias": bias}
        )
    return in_maps


def _run(inputs, trace=False, trace_kwargs=None):
    global _BUILT
    if _BUILT is None:
        _BUILT = _build_program()
    nc = _BUILT
    in_maps = _prep_inputs(**inputs)
    res = run_bass_kernel_spmd(
        nc,
        in_maps,
        core_ids=list(range(NCORES)),
        trace=trace,
        **(trace_kwargs or {}),
    )
    out = np.concatenate(
        [res.results[c]["out"].reshape(BC) for c in range(NCORES)]
    )
    return out.reshape(B, 1).astype(np.float32), res


def kernel(**inputs):
    out, _ = _run(inputs)
    return out


# revision 12
# speedup vs baseline: 1.2680x; 1.0932x over previous
"""CIN kernel: per-block L1/L2 pipeline + packed fp8 L0 + deferred scores."""

import numpy as np
import ml_dtypes
from contextlib import ExitStack

import concourse.bass as bass
import concourse.tile as tile
from concourse import bacc, mybir
from concourse.bass_utils import run_bass_kernel_spmd

F0 = 39
D = 16
B = 8192
NCORES = 8
BC = B // NCORES
N = BC * D
CH = 1024
NCHUNK = N // CH
NBLK = 20
NP0 = 6
FP16 = mybir.dt.float16
FP32 = mybir.dt.float32
FP8 = mybir.dt.float8e4

_BUILT = None


def _build_program():
    nc = bacc.Bacc(
        "TRN2",
        target_bir_lowering=False,
        debug=False,
        num_devices=NCORES,
    )

    x2_d = nc.dram_tensor("x2", [40, N], FP16, kind="ExternalInput").ap()
    xi_d = nc.dram_tensor("xi", [NP0 * 128, N], FP16, kind="ExternalInput").ap()
    xj_d = nc.dram_tensor("xj", [NP0 * 128, N], FP16, kind="ExternalInput").ap()
    f0_d = nc.dram_tensor("f0", [128, NP0 * 128], FP8, kind="ExternalInput").ap()
    f_d = [
        nc.dram_tensor(f"f{k}", [128, NBLK * 128], FP16, kind="ExternalInput").ap()
        for k in (1, 2)
    ]
    wv_d = nc.dram_tensor("wv", [128, 3], FP16, kind="ExternalInput").ap()
    bias_d = nc.dram_tensor("bias", [1, 1], FP32, kind="ExternalInput").ap()
    out_d = nc.dram_tensor("out", [1, BC], FP32, kind="ExternalOutput").ap()

    relu = mybir.ActivationFunctionType.Relu

    with tile.TileContext(nc) as tc, ExitStack() as ctx:
        const = ctx.enter_context(tc.tile_pool(name="const", bufs=1))
        pool_a = ctx.enter_context(tc.tile_pool(name="a", bufs=2))
        pool_x = ctx.enter_context(tc.tile_pool(name="x", bufs=2))
        pool_z = ctx.enter_context(tc.tile_pool(name="z", bufs=8))
        pool_h = ctx.enter_context(tc.tile_pool(name="h", bufs=3))
        pool_dt = ctx.enter_context(tc.tile_pool(name="dt", bufs=5))
        pool_r2 = ctx.enter_context(tc.tile_pool(name="r2", bufs=3))
        pool_z8 = ctx.enter_context(tc.tile_pool(name="z8", bufs=4))
        ps_curr = ctx.enter_context(tc.tile_pool(name="pcur", bufs=3, space="PSUM"))
        ps_s = ctx.enter_context(tc.tile_pool(name="ps", bufs=2, space="PSUM"))

        f0sb = const.tile([128, NP0 * 128], FP8, tag="f0sb")
        nc.sync.dma_start(f0sb[:], f0_d[:])
        fsb = []
        for k in range(2):
            f = const.tile([128, NBLK * 128], FP16, tag=f"f{k}", name=f"fc{k}")
            nc.gpsimd.dma_start(f[:, : NBLK * 64], f_d[k][:, : NBLK * 64])
            nc.gpsimd.dma_start(f[:, NBLK * 64 :], f_d[k][:, NBLK * 64 :])
            fsb.append(f)
        wv = const.tile([128, 3], FP16)
        nc.sync.dma_start(wv[:], wv_d[:])
        bias = const.tile([1, 1], FP32)
        nc.sync.dma_start(bias[:], bias_d[:])
        scores = const.tile([1, BC], FP32)

        def load_chunk(c):
            sl = slice(c * CH, (c + 1) * CH)
            xia = pool_x.tile([128, NP0 * CH], FP16, tag="xia", name=f"xia_{c}")
            xja = pool_x.tile([128, NP0 * CH], FP16, tag="xja", name=f"xja_{c}")
            src_i = xi_d[:, sl].rearrange("(b p) c -> p b c", p=128)
            src_j = xj_d[:, sl].rearrange("(b p) c -> p b c", p=128)
            nc.gpsimd.dma_start(xia[:].rearrange("p (b c) -> p b c", b=NP0), src_i)
            nc.gpsimd.dma_start(xja[:].rearrange("p (b c) -> p b c", b=NP0), src_j)
            a_all = pool_a.tile([128, NBLK * CH], FP16, tag="a", name=f"a_{c}")
            rows = x2_d[:, sl].rearrange("(j two) c -> two j c", two=2)
            for a in range(2):
                nc.gpsimd.dma_start(
                    a_all[64 * a : 64 * (a + 1)].rearrange(
                        "p (j c) -> p j c", j=NBLK
                    ),
                    rows[a : a + 1].to_broadcast([64, NBLK, CH]),
                )
            return a_all, xia, xja

        def layer_pass(c, layer, a_all, b_op, fw):
            cur = ps_curr.tile([128, CH], FP32, tag="cur", name=f"cur_{c}_{layer}")
            for k in range(NBLK):
                z = pool_z.tile([128, CH], FP16, tag="z", name=f"z_{c}_{layer}_{k}")
                nc.vector.tensor_tensor(
                    out=z[:], in0=a_all[:, k * CH : (k + 1) * CH], in1=b_op[:],
                    op=mybir.AluOpType.mult,
                )
                for sgn in range(2):
                    ssl = slice(sgn * 512, (sgn + 1) * 512)
                    nc.tensor.matmul(
                        cur[:, ssl],
                        lhsT=fw[:, k * 128 : (k + 1) * 128],
                        rhs=z[:, ssl],
                        start=(k == 0),
                        stop=(k == NBLK - 1),
                    )
            return cur

        def score_mms(sab, layer, rhs_t, rhs_k):
            for sgn in range(2):
                ssl = slice(sgn * 512, (sgn + 1) * 512)
                nc.tensor.matmul(
                    sab[32 * sgn : 32 * sgn + 1, :],
                    lhsT=wv[0:rhs_k, layer : layer + 1],
                    rhs=rhs_t[0:rhs_k, ssl],
                    start=(layer == 0), stop=(layer == 2),
                    tile_position=(0, 32 * sgn),
                )

        def do_l0(c, xia, xja):
            cur = ps_curr.tile([128, CH], FP32, tag="cur", name=f"cur0_{c}")
            z8s = []
            for q in range(3):
                z8 = pool_z8.tile([128, 2 * CH], FP8, tag="z8", name=f"z80_{c}_{q}")
                sl2 = slice(2 * q * CH, (2 * q + 2) * CH)
                nc.gpsimd.tensor_tensor(
                    out=z8[:], in0=xia[:, sl2], in1=xja[:, sl2],
                    op=mybir.AluOpType.mult,
                )
                z8s.append(z8)
            for q in range(3):
                lw = f0sb[:, q * 256 : (q + 1) * 256].rearrange(
                    "p (two m) -> p two m", two=2
                )
                zr = z8s[q][:].rearrange("p (two c) -> p two c", two=2)
                for sgn in range(2):
                    nc.tensor.matmul(
                        cur[:, sgn * 512 : (sgn + 1) * 512],
                        lhsT=lw,
                        rhs=zr[:, :, sgn * 512 : (sgn + 1) * 512],
                        start=(q == 0), stop=(q == 2),
                        perf_mode=mybir.MatmulPerfMode.DoubleRow,
                    )
            h_t = pool_h.tile([128, CH], FP16, tag="h", name=f"h_{c}")
            d_t = pool_dt.tile([64, CH], FP16, tag="d", name=f"d_{c}")
            nc.scalar.activation(h_t[0:64, :], cur[0:64, :], relu, scale=1.0 / 64)
            nc.scalar.activation(h_t[64:128, :], cur[0:64, :], relu, scale=1.0 / 64)
            nc.scalar.activation(d_t[:], cur[64:128, :], relu, scale=1.0 / 64)
            return h_t, d_t

        chunks = {}
        for cc in range(2):
            chunks[cc] = load_chunk(cc)
        state = do_l0(0, chunks[0][1], chunks[0][2])

        def emit_reduces(t, sab_t):
            for sgn in range(2):
                off = t * (CH // D) + sgn * 32
                nc.vector.tensor_reduce(
                    out=scores[0:1, off : off + 32],
                    in_=sab_t[32 * sgn : 32 * sgn + 1, :].rearrange(
                        "p (g x) -> p g x", x=D
                    ),
                    axis=mybir.AxisListType.X,
                    op=mybir.AluOpType.add,
                )

        pending_reduce = None
        for t in range(NCHUNK):
            a_all = chunks[t][0]
            h1, d_t = state
            cur1 = layer_pass(t, 1, a_all, h1, fsb[0])
            if pending_reduce is not None:
                pt, pd0, pd1, pr2 = pending_reduce
                psab = ps_s.tile([33, 512], FP32, tag="sab", name=f"sab_{pt}")
                score_mms(psab, 0, pd0, 64)
                score_mms(psab, 1, pd1, 64)
                score_mms(psab, 2, pr2, 128)
                emit_reduces(pt, psab)
            h2 = pool_h.tile([128, CH], FP16, tag="h", name=f"h2_{t}")
            d1 = pool_dt.tile([64, CH], FP16, tag="d", name=f"d1_{t}")
            nc.scalar.activation(h2[0:64, :], cur1[0:64, :], relu)
            nc.scalar.activation(h2[64:128, :], cur1[0:64, :], relu)
            nc.scalar.activation(d1[:], cur1[64:128, :], relu)
            if t + 1 < NCHUNK:
                state = do_l0(t + 1, chunks[t + 1][1], chunks[t + 1][2])
            cur2 = layer_pass(t, 2, a_all, h2, fsb[1])
            r2 = pool_r2.tile([128, CH], FP16, tag="r2", name=f"r2_{t}")
            nc.scalar.activation(r2[:], cur2[:], relu)
            if t + 2 < NCHUNK:
                chunks[t + 2] = load_chunk(t + 2)
            del chunks[t]
            pending_reduce = (t, d_t, d1, r2)

        pt, pd0, pd1, pr2 = pending_reduce
        psab = ps_s.tile([33, 512], FP32, tag="sab", name=f"sab_{pt}")
        score_mms(psab, 0, pd0, 64)
        score_mms(psab, 1, pd1, 64)
        score_mms(psab, 2, pr2, 128)
        emit_reduces(pt, psab)
        nc.vector.tensor_scalar_add(scores[:], scores[:], bias[0:1, 0:1])
        nc.sync.dma_start(out_d[:], scores[:])

    nc.compile()
    return nc


def _prep_inputs(nn_input, f0, f1, f2, w_nn, b_nn):
    nn_input = np.asarray(nn_input, dtype=np.float32)
    f0 = np.asarray(f0, dtype=np.float32)
    f1 = np.asarray(f1, dtype=np.float32)
    f2 = np.asarray(f2, dtype=np.float32)
    w_nn = np.asarray(w_nn, dtype=np.float32).reshape(-1)
    b_nn = np.asarray(b_nn, dtype=np.float32).reshape(-1)

    def pack(fp):
        blocks = fp.reshape(NBLK, 128, 128)
        return np.ascontiguousarray(
            blocks.transpose(1, 0, 2).reshape(128, NBLK * 128)
        ).astype(np.float16)

    def padf(f):
        out = np.zeros((2560, 128), np.float32)
        out[: F0 * 64] = f
        return pack(out)

    f1p, f2p = padf(f1), padf(f2)

    iu, ju = np.triu_indices(F0, k=1)
    f0r = f0.reshape(F0, F0, 128)
    w0 = np.zeros((NP0 * 128, 128), np.float32)
    w0[: len(iu)] = 2.0 * 64.0 * f0r[iu, ju]
    w0b = w0.reshape(NP0, 128, 128).transpose(1, 0, 2).reshape(128, NP0 * 128)
    f0p = np.ascontiguousarray(w0b).astype(ml_dtypes.float8_e4m3)

    wv = np.zeros((128, 3), np.float32)
    wv[0:64, 0] = 1.0 + w_nn[0:64]
    wv[0:64, 1] = 1.0 + w_nn[64:128]
    wv[:, 2] = 1.0 + w_nn[128:256]
    wv = wv.astype(np.float16)
    bias = b_nn.reshape(1, 1).astype(np.float32)

    x0 = nn_input.reshape(B, F0, D)
    in_maps = []
    for cidx in range(NCORES):
        xc = x0[cidx * BC : (cidx + 1) * BC]
        xt = xc.transpose(1, 0, 2).reshape(F0, N).astype(np.float16)
        x2h = np.zeros((40, N), np.float16)
        x2h[:F0] = xt
        xi = np.zeros((NP0 * 128, N), np.float16)
        xj = np.zeros((NP0 * 128, N), np.float16)
        xi[: len(iu)] = xt[iu]
        xj[: len(ju)] = xt[ju]
        in_maps.append(
            {"x2": x2h, "xi": xi, "xj": xj, "f0": f0p,
             "f1": f1p, "f2": f2p, "wv": wv, "bias": bias}
        )
    return in_maps


def _run(inputs, trace=False, trace_kwargs=None):
    global _BUILT
    if _BUILT is None:
        _BUILT = _build_program()
    nc = _BUILT
    in_maps = _prep_inputs(**inputs)
    res = run_bass_kernel_spmd(
        nc,
        in_maps,
        core_ids=list(range(NCORES)),
        trace=trace,
        **(trace_kwargs or {}),
    )
    out = np.concatenate(
        [res.results[c]["out"].reshape(BC) for c in range(NCORES)]
    )
    return out.reshape(B, 1).astype(np.float32), res


def kernel(**inputs):
    out, _ = _run(inputs)
    return out
